# revision 34
# baseline (speedup 1.0000x reference)
"""Trainium2 Bass kernel for nn_BinaryConv2d (B=16, C=64, H=W=256, 3x3, pad 1).

Forward semantics (STE forward values):
  act = sign(x * rd_k + rd_b)                  in {-1, 0, +1}
  bw  = scaling[co] * sign(conv_w)             scaling = mean |conv_w| per out-ch
  y   = prelu(conv2d(act, sign(conv_w)) * scaling + pr_bias0) + pr_bias1 + x

Strategy: data-parallel over batch, 2 images per core (8 cores); the two
images' 64 channels stack on the 128 SBUF partitions (block-diagonal
weights).  x ships bf16 (+pr_bias1 folded on host), y returns bf16.

v2 design (from the v1 trace: ACT 74% / DVE 73% / PE 81% busy):
  * conv: 5 DoubleRow fp8 matmuls per output row (kh0/1 pairs per kw,
    kh2 kw0/1 via an overlapping stride-1 rhs AP, kh2kw2 paired with a
    ZERO weight row).  HW-measured: every matmul streams 1 output col per
    2.4 GHz cycle regardless of perf mode (DR doubles MACs/cell, i.e.
    contraction depth, not column rate; DoublePixel/DoubleColumn lower
    and are numerically fine but give no speedup).  The PE floor is
    therefore 5*256 cycles per output row = ~141 us busy, and the trace
    shows back-to-back 110 ns matmuls with zero LDWEIGHTS bubbles.
  * matmuls are weight-major over 8-row PSUM groups (5 stationary sets)
    and a post-legalization pass drops redundant LDWEIGHTS (HW keeps the
    stationary operand across matmuls).  ERRATUM found the hard way:
    stop_tensor_calc=True on a stride-1-rhs DoubleRow matmul wedges the
    device (NRT_EXEC_UNIT_UNRECOVERABLE); stops are carried by the last
    stride-272 wdr set instead.
  * post-op: ACT drains PSUM once per group with AF.Prelu (per-partition
    alpha works on HW; Lrelu is broken), then DVE/GpSimd add the
    residual: y = prelu(s*ps + b0) + x.  One PSUM read total.
  * sign: ACT AF.Sign for ~1/3 of rows (chunks capped at 6 rows so PSUM
    drains are never stuck behind a long sign op), DVE bitwise for the
    rest ((hi&0x80)|0x38 on a stride-2 uint8 view of the bf16 x tile;
    GpSimd rejects that op form).  y stores alternate GpSimd/Sync rings.
  * 28 dummy matmuls on a memset scratch tile run during the DMA-bound
    startup so the PE_HAM clock gate (1.2 -> 2.4 GHz after ~3.4 us of
    activity) is already open when the real conv starts.
Measured: ~164 us HW exec (v1 baseline 175.5 us), abs-max rel err 5.6e-3
(gate 2e-2).  Span = ~13.5 us startup (framework preamble + first x
chunk + sign) + ~142.5 us matmul stream (at the PE floor) + ~8.5 us
drain/store/barrier tail.
"""

import os
import sys

if "/opt/trn_rl_repo" not in sys.path:
    sys.path.insert(0, "/opt/trn_rl_repo")

# bisect switches (env)
ENV_NODEDUP = os.environ.get("K_NODEDUP", "0") == "1"
ENV_SIGNACT = os.environ.get("K_SIGNACT", "0") == "1"
ENV_ADDSDVE = os.environ.get("K_ADDSDVE", "0") == "1"
ENV_STORESGS = os.environ.get("K_STORESGS", "0") == "1"
ENV_IDDRAIN = os.environ.get("K_IDDRAIN", "0") == "1"
ENV_G = int(os.environ.get("K_G", "8"))
ENV_NSTRIPS = int(os.environ.get("K_NSTRIPS", "0"))
ENV_NOK2Z = os.environ.get("K_NOK2Z", "0") == "1"
ENV_ADDSGS = os.environ.get("K_ADDSGS", "0") == "1"
ENV_K2Z2 = os.environ.get("K_K2Z2", "0") == "1"
ENV_K2SAME = os.environ.get("K_K2SAME", "0") == "1"
ENV_K2WK2 = os.environ.get("K_K2WK2", "0") == "1"
ENV_K2ORDER = os.environ.get("K_K2ORDER", "1") == "1"

from contextlib import ExitStack

import ml_dtypes
import numpy as np

import concourse.bacc as bacc
import concourse.bass as bass
import concourse.tile as tile
from concourse import mybir
from concourse.ap import AP
from concourse.bass_utils import run_bass_kernel_spmd

B, C, H, W = 16, 64, 256, 256
NCORES = 8
P = 128                      # partitions = 2 images x 64 channels

F32 = mybir.dt.float32
BF16 = mybir.dt.bfloat16
FP8 = mybir.dt.float8e4
U8 = mybir.dt.uint8
AF = mybir.ActivationFunctionType
ALU = mybir.AluOpType
DR = mybir.MatmulPerfMode.DoubleRow

APITCH = 272                 # act row pitch (bytes %16 for DoubleRow AP steps)

# Param table columns (per-partition f32 scalars)
PK, PB, PS, PSL, PB0, PZ = range(6)

# sign engine split fractions (ACT, DVE, GS) -- must sum to 1.  The
# bitwise tensor_scalar fails walrus engine checks on Pool, so GS gets
# no sign rows; it takes a share of the residual adds instead.
SIGN_FRAC = (0.32, 0.68, 0.0)
# residual-add engine pattern: groups with g % ADD_DEN < ADD_NUM add on
# DVE, the rest on GS
ADD_NUM, ADD_DEN = 3, 5
# when sign reduces to sign(xr) (k>0 uniform, badj==0), DVE/GS can use the
# bitwise trick; otherwise all sign rows go through ACT.  kernel() sets it.
BITWISE_OK = True
# ACT sign via float immediates when rd_k/badj are channel-uniform
SIGN_IMM = None
# sim mode: replace Prelu with Identity (interp lacks Prelu); HW runs Prelu
SIM_SAFE = False

G8 = ENV_G                   # output rows per PSUM group
# strip heights (sum == H, all % 8 == 0): small first strip for fast
# pipeline fill, small last strip for a short drain/store tail
STRIP_HS = [32, 40, 40, 40, 40, 40, 16, 8]
if ENV_NSTRIPS:
    STRIP_HS = STRIP_HS[:ENV_NSTRIPS]


def dedupe_ldweights(nc):
    """Drop InstLdweights that reload the identical stationary operand
    (the PE keeps weights across matmuls; tile legalization emits one
    LDWEIGHTS per matmul unconditionally).  Only drops LDWs with no
    waits/updates; resets tracking on any other PE-queue instruction."""
    pe_engine = None
    ndrop = 0
    for blk in nc.main_func.blocks:
        last_sig = None
        keep = []
        for inst in blk.instructions:
            if isinstance(inst, mybir.InstLdweights):
                if pe_engine is None:
                    pe_engine = inst.engine
                sig = (str(inst.ins[0]), str(inst.perf_mode),
                       str(inst.is_transpose), str(inst.tile_position),
                       str(inst.tile_size))
                si = inst.sync_info
                clean = si is None or (not si.on_wait and not si.on_update)
                if sig == last_sig and clean:
                    ndrop += 1
                    continue
                last_sig = sig
                keep.append(inst)
            else:
                if not isinstance(inst, mybir.InstMatmult):
                    if pe_engine is not None and inst.engine == pe_engine:
                        last_sig = None
                keep.append(inst)
        blk.instructions[:] = keep
    return ndrop


def _split_rows(nrows):
    """Partition [0, nrows) into per-engine chunk lists by SIGN_FRAC.
    Returns {engine_idx: [(c0, sz), ...]} with chunks <= 12 rows."""
    if not BITWISE_OK:
        bounds = [0, nrows, nrows]
    else:
        a = int(round(nrows * SIGN_FRAC[0]))
        d = int(round(nrows * (SIGN_FRAC[0] + SIGN_FRAC[1])))
        bounds = [a, d, nrows]
    caps = (6, 8, 12)
    out = {}
    lo = 0
    for ei, hi in enumerate(bounds):
        chunks = []
        c = lo
        while c < hi:
            sz = min(caps[ei], hi - c)
            chunks.append((c, sz))
            c += sz
        out[ei] = chunks
        lo = hi
    return out


def _emit(tc, nc, x_d, w_d, p_d, y_d):
    x3 = x_d.rearrange("p (h w) -> p h w", w=W)
    y3 = y_d.rearrange("p (h w) -> p h w", w=W)

    with ExitStack() as ctx:
        consts = ctx.enter_context(tc.tile_pool(name="consts", bufs=1))
        xpool = ctx.enter_context(tc.tile_pool(name="xpool", bufs=3))
        apool = ctx.enter_context(tc.tile_pool(name="apool", bufs=2))
        ypool = ctx.enter_context(tc.tile_pool(name="ypool", bufs=4))
        tpool = ctx.enter_context(tc.tile_pool(name="tpool", bufs=4))
        pspool = ctx.enter_context(tc.tile_pool(
            name="pspool", bufs=16 // G8, space="PSUM"))

        pt = consts.tile([P, 8], F32)
        nc.sync.dma_start(out=pt, in_=p_d)
        # [kw, delta(kh 0/1), m] DoubleRow pairs
        wdr = consts.tile([P, 3, 2, 128], FP8)
        nc.scalar.dma_start(out=wdr[:, 0], in_=w_d[:, :256].rearrange(
            "p (d m) -> p d m", d=2))
        nc.scalar.dma_start(out=wdr[:, 1:], in_=w_d[:, 256:768].rearrange(
            "p (k d m) -> p k d m", k=2, d=2))
        # kh=2: [delta(kw 0/1), m] DoubleRow
        wk2 = consts.tile([P, 2, 128], FP8)
        nc.scalar.dma_start(out=wk2, in_=w_d[:, 768:1024].rearrange(
            "p (d m) -> p d m", d=2))
        # kh=2,kw=2 zero-paired: row d=0 is zeros, d=1 the real weights
        wn2z = consts.tile([P, 2, 128], FP8)
        nc.scalar.dma_start(out=wn2z, in_=w_d[:, 1024:1280].rearrange(
            "p (d m) -> p d m", d=2))

        H0S = [sum(STRIP_HS[:i]) for i in range(len(STRIP_HS))]
        NST = len(STRIP_HS)
        HSMAX = max(STRIP_HS)

        def strip_rows(s):
            h0 = H0S[s]
            row_lo = max(h0 - 1, 0)
            row_hi = min(h0 + STRIP_HS[s] + 1, H)
            return h0, row_lo, row_hi, row_lo - (h0 - 1)

        def load_strip(s):
            """DMA the x strip (rows h0-1 .. h0+hs; tile row a <-> global
            h0-1+a) and memset the act padding."""
            h0, row_lo, row_hi, r0 = strip_rows(s)
            nr = row_hi - row_lo
            xs = xpool.tile([P, HSMAX + 2, W], BF16, name="xs")
            if s == 0:
                # first transfer covers exactly the rows sign chunk 1 needs
                bounds = [0, 4]
                while bounds[-1] < nr:
                    bounds.append(min(bounds[-1] + 10, nr))
            else:
                bounds = [row_lo, row_lo + nr // 2, row_lo + nr]
            for idx, (a, b) in enumerate(zip(bounds, bounds[1:])):
                if b > a:
                    # first startup chunk issues from the GS ring so it
                    # doesn't queue behind the param-table load
                    eng = nc.gpsimd if (s == 0 and idx == 0) else nc.sync
                    eng.dma_start(out=xs[:, a - (h0 - 1):b - (h0 - 1), :],
                                  in_=x3[:, a:b, :])
            act = apool.tile([P, HSMAX + 2, APITCH], FP8, name="act")
            nrows = STRIP_HS[s] + 2
            nc.gpsimd.memset(act[:, :nrows, 0:1], 0.0)
            nc.gpsimd.memset(act[:, :nrows, W + 1:W + 2], 0.0)
            if ENV_K2Z2:
                nc.gpsimd.memset(act[:, :nrows, W + 2:W + 3], 0.0)
            if s == 0:
                nc.gpsimd.memset(act[:, 0:1, :], 0.0)
            if s == NST - 1:
                nc.gpsimd.memset(act[:, nrows - 1:nrows, :], 0.0)
            return xs, act

        def sign_chunk(s, xs, act, c0, sz, eng):
            """Binarize rows [c0, c0+sz) of the strip's x tile into the
            zero-padded act tile on the given engine (0=ACT 1=DVE 2=GS)."""
            _, row_lo, row_hi, r0 = strip_rows(s)
            lo = r0 + c0
            hi = min(lo + sz, r0 + (row_hi - row_lo))
            if hi <= lo:
                return
            if eng == 0:
                if SIGN_IMM is not None:
                    nc.scalar.activation(
                        act[:, lo:hi, 1:W + 1], xs[:, lo:hi, :], AF.Sign,
                        bias=SIGN_IMM[1], scale=SIGN_IMM[0],
                    )
                else:
                    nc.scalar.activation(
                        act[:, lo:hi, 1:W + 1], xs[:, lo:hi, :], AF.Sign,
                        bias=pt[:, PB:PB + 1], scale=pt[:, PK:PK + 1],
                    )
            else:
                xu = xs.bitcast(U8).rearrange(
                    "p r (n two) -> p r n two", two=2)
                au = act.bitcast(U8)
                e = nc.vector if eng == 1 else nc.gpsimd
                e.tensor_scalar(
                    au[:, lo:hi, 1:W + 1], xu[:, lo:hi, :, 1],
                    0x80, 0x38, ALU.bitwise_and, ALU.bitwise_or,
                )

        def kh2_pair_rhs(act, row, col0):
            """[P, 2, W] rhs with both free strides 1: element (d, m) reads
            padded act col col0+d+m of `row` (DR tap pairing along kw)."""
            base = act[:, row, col0:col0 + W]
            return AP(base.tensor, base.offset,
                      [list(base.ap[0]), [1, 2], [1, W]])

        def conv_group(act, ps8, r0, gr):
            """All matmuls for gr output rows (act tap rows r0..r0+gr+1),
            weight-major: 5 stationary sets, gr DR matmuls each."""
            if ENV_K2ORDER:
                # stop carried by the last wdr set (stride-272 DR);
                # stop=True on a stride-1 DR matmul wedges the device
                for kw in range(2):
                    for i in range(gr):
                        nc.tensor.matmul(
                            ps8[:, i, :], lhsT=wdr[:, kw, :, :],
                            rhs=act[:, r0 + i:r0 + i + 2, kw:kw + W],
                            start=(kw == 0 and i % 2 == 0),
                            stop=False, perf_mode=DR,
                        )
                for i in range(gr):
                    nc.tensor.matmul(
                        ps8[:, i, :], lhsT=wk2,
                        rhs=kh2_pair_rhs(act, r0 + i + 2, 0),
                        start=False, stop=False, perf_mode=DR,
                    )
                for i in range(gr):
                    nc.tensor.matmul(
                        ps8[:, i, :], lhsT=wn2z,
                        rhs=kh2_pair_rhs(act, r0 + i + 2, 1),
                        start=False, stop=False, perf_mode=DR,
                    )
                for i in range(gr):
                    nc.tensor.matmul(
                        ps8[:, i, :], lhsT=wdr[:, 2, :, :],
                        rhs=act[:, r0 + i:r0 + i + 2, 2:2 + W],
                        start=False, stop=(i % 2 == 1), perf_mode=DR,
                    )
                return
            for kw in range(3):
                for i in range(gr):
                    nc.tensor.matmul(
                        ps8[:, i, :], lhsT=wdr[:, kw, :, :],
                        rhs=act[:, r0 + i:r0 + i + 2, kw:kw + W],
                        start=(kw == 0 and i % 2 == 0),
                        stop=False, perf_mode=DR,
                    )
            for i in range(gr):
                nc.tensor.matmul(
                    ps8[:, i, :], lhsT=wk2,
                    rhs=kh2_pair_rhs(act, r0 + i + 2, 0),
                    start=False, stop=False, perf_mode=DR,
                )
            if ENV_K2Z2:
                for i in range(gr):
                    nc.tensor.matmul(
                        ps8[:, i, :], lhsT=wn2z,
                        rhs=kh2_pair_rhs(act, r0 + i + 2, 2),
                        start=False, stop=(i % 2 == 1), perf_mode=DR,
                    )
            elif ENV_NOK2Z:
                for h in range(gr // 2):
                    nc.tensor.matmul(
                        ps8[:, 2 * h:2 * h + 2, :], lhsT=wn2z[:, 1, :],
                        rhs=act[:, r0 + 2 * h + 2:r0 + 2 * h + 4, 2:2 + W],
                        start=False, stop=True,
                    )
            else:
                cz = 0 if (ENV_K2SAME or ENV_K2WK2) else 1
                wz = wk2 if ENV_K2WK2 else wn2z
                for i in range(gr):
                    nc.tensor.matmul(
                        ps8[:, i, :], lhsT=wz,
                        rhs=kh2_pair_rhs(act, r0 + i + 2, cz),
                        start=False, stop=(i % 2 == 1), perf_mode=DR,
                    )

        def post_group(g, s, xs, ps8, r0, gr):
            """Prelu drain (ACT), residual add (DVE), store (GS/Sync)."""
            t8 = tpool.tile([P, gr, W], BF16, name="t")
            fn = AF.Identity if (SIM_SAFE or ENV_IDDRAIN) else AF.Prelu
            nc.scalar.activation(
                t8, ps8, fn,
                bias=pt[:, PB0:PB0 + 1], scale=pt[:, PS:PS + 1],
                alpha=pt[:, PSL:PSL + 1],
            )
            y8 = ypool.tile([P, gr, W], BF16, name="y")
            adder = (nc.vector if (not ENV_ADDSGS and
                                    (ENV_ADDSDVE or g % ADD_DEN < ADD_NUM))
                     else nc.gpsimd)
            adder.tensor_tensor(y8, t8, xs[:, r0 + 1:r0 + 1 + gr, :],
                                ALU.add)
            h0 = H0S[s]
            eng = nc.gpsimd if (ENV_STORESGS or g % 2 == 0) else nc.sync
            eng.dma_start(out=y3[:, h0 + r0:h0 + r0 + gr, :], in_=y8)

        # PE warmup: the HAM clock gate needs ~3.4us of sustained PE
        # activity to lift the PE from 1.2 to 2.4 GHz; run dummy matmuls
        # on a memset scratch tile during the DMA-bound startup so the
        # real conv starts at full clock.  The scratch PSUM tile is
        # start/stopped so group 0 can reuse the buffer cleanly.
        wz = consts.tile([P, 256], FP8)
        nc.gpsimd.memset(wz, 0.0)
        pswt = pspool.tile([P, G8, W], F32, name="ps")
        psw = pswt[:, 0, :]
        NWARM = 28
        for k in range(NWARM):
            nc.tensor.matmul(psw, lhsT=wz[:, 0:128], rhs=wz,
                             start=(k == 0), stop=(k == NWARM - 1))

        # strip 0 signs rows 0 .. STRIP_HS[0]+1 (no top halo row); spread
        # progressive chunks round-robin over DVE, GS, ACT so the first
        # conv group (act rows 0..9) unblocks as soon as x lands
        cur = load_strip(0)
        n0 = STRIP_HS[0] + 1
        if BITWISE_OK:
            s0_chunks = [(0, 4, 1), (4, 5, 0), (9, 5, 1), (14, 7, 0),
                         (21, 6, 1), (27, max(n0 - 27, 0), 0)]
        else:
            s0_chunks = [(0, 4, 0), (4, 5, 0), (9, 5, 0), (14, 7, 0),
                         (21, 6, 0), (27, max(n0 - 27, 0), 0)]
        for c0, sz, eng in s0_chunks:
            if sz > 0:
                sign_chunk(0, *cur, c0, sz, eng)
        nxt = None
        g = 0                      # global 8-row group index
        for s in range(NST):
            HS_S = STRIP_HS[s]
            # the last strip runs 4-row groups: its drain/add/store chain
            # is on the critical path after the final matmul
            GR_S = 4 if s == NST - 1 else G8
            NG = HS_S // GR_S
            xs, act = cur
            # next-strip sign chunks, interleaved into each engine queue
            sign_plan = {}
            if s + 1 < NST:
                per_eng = _split_rows(STRIP_HS[s + 1] + 2)
                for ei, chunks in per_eng.items():
                    for j, (c0, sz) in enumerate(chunks):
                        # schedule so chunks finish by strip end; early
                        # chunks first (rows needed by group 0 of s+1)
                        emit_k = max(1, NG - (len(chunks) - j) - 1)
                        sign_plan.setdefault(emit_k, []).append(
                            (c0, sz, ei))
            for k in range(NG):
                ps8 = pspool.tile([P, G8, W], F32, name="ps")
                if GR_S != G8:
                    ps8 = ps8[:, 0:GR_S, :]
                conv_group(act, ps8, GR_S * k, GR_S)
                if k == min(1, NG - 1) and s + 1 < NST:
                    nxt = load_strip(s + 1)   # loads overlap this strip
                post_group(g, s, xs, ps8, GR_S * k, GR_S)
                for c0, sz, ei in sign_plan.pop(k, ()):
                    sign_chunk(s + 1, *nxt, c0, sz, ei)
                g += 1
            for emit_k in sorted(sign_plan):
                for c0, sz, ei in sign_plan[emit_k]:
                    sign_chunk(s + 1, *nxt, c0, sz, ei)
            cur = nxt


def build_nc():
    nc = bacc.Bacc("TRN2", target_bir_lowering=False, debug=False,
                   num_devices=NCORES)
    x_d = nc.dram_tensor("xin", [P, H * W], BF16, kind="ExternalInput").ap()
    w_d = nc.dram_tensor("wp", [P, 10 * 128], FP8, kind="ExternalInput").ap()
    p_d = nc.dram_tensor("pp", [P, 8], F32, kind="ExternalInput").ap()
    y_d = nc.dram_tensor("yout", [P, H * W], BF16, kind="ExternalOutput").ap()
    with tile.TileContext(nc) as tc:
        _emit(tc, nc, x_d, w_d, p_d, y_d)
    if not ENV_NODEDUP:
        nd = dedupe_ldweights(nc)
        assert nd > 0
    nc.compile()
    return nc


_NC_CACHE = {}


def _get_nc():
    key = (BITWISE_OK, SIGN_IMM, SIM_SAFE, SIGN_FRAC, ADD_NUM, ADD_DEN, G8)
    if key not in _NC_CACHE:
        _NC_CACHE[key] = build_nc()
    return _NC_CACHE[key]


def make_inputs(x, rd_k, rd_b, beta, conv_w, pr_bias0, prelu_w, pr_bias1):
    """Host-side prep: per-channel param table, packed sign weights, shards."""
    k = np.asarray(rd_k, np.float32).reshape(C)
    b = np.asarray(rd_b, np.float32).reshape(C)
    s = np.mean(np.abs(np.asarray(conv_w, np.float32)), axis=(1, 2, 3))
    b0 = np.asarray(pr_bias0, np.float32).reshape(C)
    slope = np.asarray(prelu_w, np.float32).reshape(C)
    b1 = np.asarray(pr_bias1, np.float32).reshape(C)
    # pr_bias1 is folded into the residual input x' = x + b1; the sign
    # threshold compensates: sign(k*x + b) == sign(k*x' + (b - k*b1))
    badj = b - k * b1
    global SIGN_IMM, BITWISE_OK
    if np.all(k == k[0]) and np.all(badj == badj[0]):
        SIGN_IMM = (float(k[0]), float(badj[0]))
    else:
        SIGN_IMM = None
    BITWISE_OK = bool(np.all(k > 0) and np.all(badj == 0.0)
                      and not ENV_SIGNACT)
    cols = np.stack([k, badj, s, slope, b0,
                     np.zeros(C, np.float32), np.zeros(C, np.float32),
                     np.zeros(C, np.float32)], axis=1)
    pp = np.concatenate([cols, cols], axis=0).astype(np.float32)  # [128, 8]

    sw = np.sign(np.asarray(conv_w, np.float32)).astype(np.float32)

    def blockdiag(kh, kw):
        S = sw[:, :, kh, kw].T  # [ci, co]
        out = np.zeros((P, P), np.float32)
        out[0:C, 0:C] = S
        out[C:P, C:P] = S
        return out

    wp = np.zeros((P, 10, 128), np.float32)
    for kw in range(3):            # [kw, delta(kh 0/1), m] DoubleRow pairs
        for d in range(2):
            wp[:, kw * 2 + d, :] = blockdiag(d, kw)
    for d in range(2):             # kh=2: [delta(kw 0/1), m] DoubleRow
        wp[:, 6 + d, :] = blockdiag(2, d)
    if ENV_K2Z2:
        wp[:, 8, :] = blockdiag(2, 2)  # real weights first (d=0)
        wp[:, 9, :] = 0.0              # zero pair row (d=1)
    else:
        wp[:, 8, :] = 0.0              # zero-pair row (d=0) for kh=2,kw=2
        wp[:, 9, :] = blockdiag(2, 2)  # kh=2,kw=2 real weights (d=1)
    wp = np.ascontiguousarray(wp.reshape(P, 10 * 128)).astype(
        mybir.dt.np(FP8))

    xr = np.asarray(x, np.float32) + b1[None, :, None, None]
    xr = xr.astype(ml_dtypes.bfloat16)
    in_maps = []
    for c in range(NCORES):
        xc = np.ascontiguousarray(xr[2 * c:2 * c + 2]).reshape(P, H * W)
        in_maps.append({"xin": xc, "wp": wp, "pp": pp})
    return in_maps


def kernel(x, rd_k, rd_b, beta, conv_w, pr_bias0, prelu_w, pr_bias1):
    in_maps = make_inputs(x, rd_k, rd_b, beta, conv_w, pr_bias0, prelu_w,
                          pr_bias1)
    nc = _get_nc()
    res = run_bass_kernel_spmd(nc, in_maps, core_ids=list(range(NCORES)))
    y = np.empty((B, C, H, W), np.float32)
    for c in range(NCORES):
        y[2 * c:2 * c + 2] = np.asarray(
            res.results[c]["yout"]).astype(np.float32).reshape(2, C, H, W)
    return y
